# revision 29
# baseline (speedup 1.0000x reference)
"""Trainium2 Bass kernel for nn_Attention_40407052320883 (sparse GQA attention).

Sharding: B(2) x KV(4) = 8 independent attention problems, one per NeuronCore.
Each core computes, for its (batch b, kv-group g):
  - qT/kT/vT projections in bf16 (weights stationary, x^T moving)
  - RMSNorm via squared-copy + GPSIMD partition all-reduce + ACT rsqrt;
    RoPE via swap-matrix matmul and DVE combines (tables hold norm scales)
  - transposed-S attention: S^T = K Q^T per 128-key block at exact sliding
    window widths, softcap tanh -> exp on ACT, multiplicative edge masks on
    DVE; PV runs with p stationary and [v | 1] moving so each 129-col matmul
    accumulates ctx[q, d] AND the softmax denominator in one pass
  - per-partition normalize (denominator is per-q-partition), PE transpose
    back to [d, q], out-projection out^T = Wo_r^T ctx^T
Host: transposes x, slices weights, builds RoPE tables, converts to bf16,
sums the 4 per-kv partial out^T per batch and transposes back.
"""

import numpy as np

B, S, E = 2, 2048, 2048
H, KV, D = 16, 4, 128
G = H // KV
WIN = 1024
CAP = 50.0
EPS = 1e-6
THETA = 10000.0
SCALE = D ** -0.5

N_CORES = 8
EC = E // 128          # 16 e-chunks
ST = S // 128          # 16 s-tiles
NQ = S // 512          # 4 s-quarters

# sliding-window block geometry: for q-chunk j (512 wide) and k-block m (128
# wide), d0 = 4j - m.  full blocks: 1<=d0<=4.  partial causal: -3<=d0<=0.
# partial window: 5<=d0<=8.  exact column ranges (within the 512-wide q
# chunk) that can contain nonzero weights:
_D0_EXACT = {
    -3: (384, 512), -2: (256, 512), -1: (128, 512), 0: (0, 512),
    5: (0, 512), 6: (0, 384), 7: (0, 256), 8: (0, 128),
}
_D0_MASK_IDX = {-3: 0, -2: 1, -1: 2, 0: 3, 5: 4, 6: 5, 7: 6, 8: 7}


def _build_module(nrep=1, debug=False):
    import contextlib
    import concourse.bacc as bacc
    import concourse.tile as tile
    import concourse.mybir as mybir
    from concourse.bass import bass_isa

    f32 = mybir.dt.float32
    bf16 = mybir.dt.bfloat16
    MUL = mybir.AluOpType.mult
    ADD = mybir.AluOpType.add
    Act = mybir.ActivationFunctionType
    RADD = bass_isa.ReduceOp.add

    nc = bacc.Bacc(
        "TRN2", target_bir_lowering=False, debug=False, enable_asserts=False,
        num_devices=N_CORES,
    )

    xT = nc.dram_tensor("xT", [EC, 128, S], bf16, kind="ExternalInput").ap()
    wqkv = nc.dram_tensor("wqkv", [EC, 128, 768], bf16, kind="ExternalInput").ap()
    wo = nc.dram_tensor("wo", [G, 128, E], bf16, kind="ExternalInput").ap()
    tabs = nc.dram_tensor("tabs", [4, 128, S], bf16, kind="ExternalInput").ap()
    masks = nc.dram_tensor("masks", [8, 128, 512], bf16, kind="ExternalInput").ap()
    identb = nc.dram_tensor("identb", [128, 128], bf16, kind="ExternalInput").ap()
    swapb = nc.dram_tensor("swapb", [128, 128], bf16, kind="ExternalInput").ap()
    onesm = nc.dram_tensor("onesm", [128, 128], bf16, kind="ExternalInput").ap()
    outT = nc.dram_tensor("outT", [E, S], bf16, kind="ExternalOutput").ap()
    if debug:
        dbg_q = nc.dram_tensor("dbg_q", [128, G, S], bf16, kind="ExternalOutput").ap()
        dbg_k = nc.dram_tensor("dbg_k", [128, S], bf16, kind="ExternalOutput").ap()
        dbg_v = nc.dram_tensor("dbg_v", [128, ST, 129], bf16, kind="ExternalOutput").ap()
        dbg_ctx = nc.dram_tensor("dbg_ctx", [128, G, S], bf16, kind="ExternalOutput").ap()

    c1 = float(SCALE / CAP)

    with tile.TileContext(nc) as tc:
      with nc.allow_low_precision(reason="bf16 activations/outputs by design"), \
           (tc.For_i(0, nrep, 1) if nrep > 1 else contextlib.nullcontext()):
        with (
            tc.tile_pool(name="consts", bufs=1) as consts,
            tc.tile_pool(name="mask", bufs=1) as m_pool,
            tc.tile_pool(name="qkv", bufs=1) as qkv_pool,
            tc.tile_pool(name="wop", bufs=1) as wo_pool,
            tc.tile_pool(name="ctxp", bufs=1) as ctx_pool,
            tc.tile_pool(name="scores", bufs=1) as a_pool,
            tc.tile_pool(name="oev", bufs=3) as ob_pool,
        ):
            mask_sb = m_pool.tile([128, 8, 512], bf16, tag="masks")
            eps_sb = consts.tile([128, 1], f32, tag="eps")
            nc.gpsimd.memset(eps_sb[:, :], float(EPS))
            identb_sb = consts.tile([128, 128], bf16, tag="identb")
            swapb_sb = consts.tile([128, 128], bf16, tag="swapb")
            onesm_sb = consts.tile([128, 128], bf16, tag="onesm")

            # per-quarter tiles so attention reads depend only on the
            # quarters that produced them
            qT_q = [qkv_pool.tile([128, G, 512], bf16, tag=f"qT{t}",
                                  name=f"qT{t}") for t in range(NQ)]
            kT_q = [qkv_pool.tile([128, 512], bf16, tag=f"kT{t}",
                                  name=f"kT{t}") for t in range(NQ)]
            # v blocks carry a 129th all-ones column: the PV matmul then
            # accumulates the softmax denominator alongside ctx
            v_q = [qkv_pool.tile([128, 4, 129], bf16, tag=f"v{t}",
                                 name=f"v{t}") for t in range(NQ)]
            for t in range(NQ):
                nc.gpsimd.memset(v_q[t][:, :, 128:129], 1.0)

            wo_sb = wo_pool.tile([128, G, E], bf16, tag="wo")
            ctx_sb = ctx_pool.tile([128, G, S], bf16, tag="ctx")

            def blocks_of(j):
                m_lo, m_hi = max(0, 4 * j - 8), min(ST - 1, 4 * j + 3)
                blocks = [(m, 4 * j - m) + _D0_EXACT.get(4 * j - m, (0, 512))
                          for m in range(m_lo, m_hi + 1)]
                blocks.sort(key=lambda b: -b[1])  # d0 descending
                return blocks

            def pairs_of(j):
                blocks = blocks_of(j)
                return [blocks[i:i + 2] for i in range(0, len(blocks), 2)]

            # p slot per (j, h, block): filled by score_pair, consumed by the
            # per-chunk PV accumulation of pv_norm
            p_of = {}
            pending = []   # out-projection units, drained in phase B

            def score_pair(j, h, pair, stp, emit=None):
                u0 = min(b[2] for b in pair)
                u1 = max(b[3] for b in pair)
                st_ps = stp.tile([128, 1024], f32, tag="st", name="st_ps")
                p_sb = a_pool.tile([128, 1024], bf16, tag="p", bufs=10,
                                   name="p_sb")
                t_sb = a_pool.tile([128, 1024], bf16, tag="t", bufs=2,
                                   name="t_sb")
                for k, (m, d0, w0, w1) in enumerate(pair):
                    # write the pair's union range so tanh/exp can run one
                    # strided op over clean psum; PV consumes exact chunks
                    nc.tensor.matmul(
                        st_ps[:, k * 512 + u0:k * 512 + u1],
                        kT_q[m // 4][:, (m % 4) * 128:(m % 4 + 1) * 128],
                        qT_q[j][:, h, u0:u1],
                        start=True, stop=True)
                if emit is not None:
                    emit()
                st_v = st_ps[:, :].rearrange(
                    "p (k f) -> p k f", k=2)[:, :, u0:u1]
                t_v = t_sb[:, :].rearrange(
                    "p (k f) -> p k f", k=2)[:, :, u0:u1]
                p_v = p_sb[:, :].rearrange(
                    "p (k f) -> p k f", k=2)[:, :, u0:u1]
                nc.scalar.activation(t_v, st_v, Act.Tanh, scale=c1)
                nc.scalar.activation(p_v, t_v, Act.Exp, scale=float(CAP))
                for k, (m, d0, w0, w1) in enumerate(pair):
                    if d0 in _D0_MASK_IDX:
                        mi = _D0_MASK_IDX[d0]
                        psl = slice(k * 512 + u0, k * 512 + u1)
                        nc.vector.tensor_tensor(
                            p_sb[:, psl], p_sb[:, psl],
                            mask_sb[:, mi, u0:u1], op=MUL)
                    p_of[(j, h, m)] = (p_sb, k)

            def pv_norm(j, h, ctx_psum_pool, tr_pool, tr_tag):
                blocks = blocks_of(j)
                cov = {m: range(w0 // 128, w1 // 128)
                       for (m, d0, w0, w1) in blocks}
                ctx_ps = ctx_psum_pool.tile([128, 4, 256], f32, tag="ctx",
                                            name="ctx_ps")
                for qc in range(4):
                    todo = [m for m in cov if qc in cov[m]]
                    for i, m in enumerate(todo):
                        p_sb, k = p_of[(j, h, m)]
                        nc.tensor.matmul(
                            ctx_ps[:, qc, 0:129],
                            p_sb[:, k * 512 + qc * 128:k * 512 + (qc + 1) * 128],
                            v_q[m // 4][:, m % 4, :],
                            start=(i == 0), stop=(i == len(todo) - 1))
                # normalize (denominator sits in col 128, one value per q
                # partition), transpose back to [d, q]
                big = tr_pool.tile([128, 512], bf16, tag=tr_tag, name="big")
                rec4 = a_pool.tile([128, 4], f32, tag="rec4", bufs=2,
                                   name="rec4")
                nc.vector.reciprocal(
                    rec4[:, :],
                    ctx_ps[:, :, 128:129].rearrange("p a b -> p (a b)"))
                for qc in range(4):
                    ctxn = a_pool.tile([128, 128], bf16, tag="cn", bufs=3,
                                       name="ctxn")
                    nc.vector.tensor_scalar(
                        out=ctxn[:, :], in0=ctx_ps[:, qc, 0:128],
                        scalar1=rec4[:, qc:qc + 1], scalar2=None, op0=MUL)
                    nc.tensor.transpose(
                        big[:, qc * 128:(qc + 1) * 128],
                        ctxn[:, :], identb_sb[:, :])
                nc.vector.tensor_copy(
                    ctx_sb[:, h, j * 512:(j + 1) * 512], big[:, :])

            def emit_oproj(pool, tag):
                ec, jj = pending.pop(0)
                esl = slice(ec * 128, (ec + 1) * 128)
                po = pool.tile([128, 512], f32, tag=tag, name="po")
                for hh in range(G):
                    nc.tensor.matmul(
                        po[:, :], wo_sb[:, hh, esl],
                        ctx_sb[:, hh, jj * 512:(jj + 1) * 512],
                        start=(hh == 0), stop=(hh == G - 1))
                ob = ob_pool.tile([128, 512], bf16, tag="ob", name="ob")
                nc.vector.tensor_copy(ob[:, :], po[:, :])
                nc.sync.dma_start(
                    outT[esl, jj * 512:(jj + 1) * 512], ob[:, :])

            # ======== phase A: projections qt0-3 with attention j0-2
            # interleaved one quarter behind ==============================
            with (
                tc.tile_pool(name="wq", bufs=1) as w_pool,
                tc.tile_pool(name="xq", bufs=2) as x_pool,
                tc.tile_pool(name="tab", bufs=2) as tab_pool,
                tc.tile_pool(name="p1t", bufs=2) as t_pool,
                tc.tile_pool(name="p1ps", bufs=2, space="PSUM") as ps1,
                tc.tile_pool(name="p1aux", bufs=1, space="PSUM") as ps_aux,
                tc.tile_pool(name="ast", bufs=1, space="PSUM") as st1_pool,
                tc.tile_pool(name="actx", bufs=1, space="PSUM") as ps_ctx1,
            ):
                wq_sb = w_pool.tile([128, EC, 768], bf16, tag="wqkv")

                work = []      # deferred attention closures (scores+pv)

                def drain(k):
                    for _ in range(k):
                        if work:
                            work.pop(0)()

                for qt in range(NQ):
                    sl = slice(qt * 512, (qt + 1) * 512)
                    if qt == 0:
                        # first two chains' weights, then x halves, then the
                        # rest of the weights / consts
                        nc.sync.dma_start(
                            wq_sb[:, :, 0:256],
                            wqkv[:, :, 0:256].rearrange("e p f -> p e f"))
                    xa = x_pool.tile([128, EC // 2, 512], bf16, tag="xqa")
                    nc.sync.dma_start(xa[:, :, :],
                                      xT[0:EC // 2, :, sl]
                                      .rearrange("e p s -> p e s"))
                    xb = x_pool.tile([128, EC // 2, 512], bf16, tag="xqb")
                    nc.sync.dma_start(xb[:, :, :],
                                      xT[EC // 2:EC, :, sl]
                                      .rearrange("e p s -> p e s"))
                    rhs = [xa[:, ec, :] if ec < EC // 2
                           else xb[:, ec - EC // 2, :] for ec in range(EC)]
                    if qt == 0:
                        nc.sync.dma_start(
                            wq_sb[:, :, 256:512],
                            wqkv[:, :, 256:512].rearrange("e p f -> p e f"))
                        nc.sync.dma_start(swapb_sb[:, :], swapb[:, :])
                        nc.sync.dma_start(identb_sb[:, :], identb[:, :])
                        nc.sync.dma_start(onesm_sb[:, :], onesm[:, :])
                        nc.sync.dma_start(
                            wq_sb[:, :, 512:768],
                            wqkv[:, :, 512:768].rearrange("e p f -> p e f"))
                    if qt == 1:
                        nc.sync.dma_start(
                            mask_sb[:, :, :],
                            masks[:, :, :].rearrange("a p f -> p a f"))
                    if qt == 3:
                        nc.sync.dma_start(
                            wo_sb[:, :, :],
                            wo[:, :, :].rearrange("g p e -> p g e"))
                    tab_sb = tab_pool.tile([128, 4, 512], bf16, tag="tabs")
                    nc.sync.dma_start(
                        tab_sb[:, :, :],
                        tabs[:, :, sl].rearrange("a p f -> p a f"))
                    del sl

                    if qt > 0:
                        j = qt - 1
                        hs = range(G) if qt < 3 else range(2)
                        for h in hs:
                            for pair in pairs_of(j):
                                work.append(
                                    (lambda jj, hh, pp:
                                     lambda: score_pair(jj, hh, pp, st1_pool)
                                     )(j, h, pair))
                            work.append(
                                (lambda jj, hh:
                                 lambda: pv_norm(jj, hh, ps_ctx1, ps_aux,
                                                 "trx"))(j, h))

                    fin = []   # (t1, dst) for this quarter's rms finish
                    var_all = t_pool.tile([128, 5, 512], bf16, tag="var",
                                          name="var_all")
                    for ch in range(6):
                        ps = ps1.tile([128, 512], f32, tag="pqkv")
                        for ec in range(EC):
                            nc.tensor.matmul(
                                ps[:, :],
                                wq_sb[:, ec, ch * 128:(ch + 1) * 128],
                                rhs[ec],
                                start=(ec == 0), stop=(ec == EC - 1),
                            )
                        if ch == 5:
                            # v: evacuate + transpose back to [s, d]
                            vt = t_pool.tile([128, 512], bf16, tag="vT")
                            nc.scalar.copy(vt[:, :], ps[:, :])
                            vtr = ps_aux.tile([128, 512], bf16, tag="trx",
                                              name="vtr")
                            for t4 in range(4):
                                nc.tensor.transpose(
                                    vtr[:, t4 * 128:(t4 + 1) * 128],
                                    vt[:, t4 * 128:(t4 + 1) * 128],
                                    identb_sb[:, :])
                            nc.vector.tensor_copy(
                                v_q[qt][:, :, 0:128],
                                vtr[:, :].rearrange("p (a b) -> p a b", a=4))
                        else:
                            # rope on the raw q; rms 1/sqrt scale is applied
                            # in a batched finish at quarter end (keeps Sqrt
                            # table loads away from the Tanh/Exp stream)
                            qraw = t_pool.tile([128, 512], bf16, tag="qraw")
                            nc.scalar.copy(qraw[:, :], ps[:, :])
                            sq = t_pool.tile([128, 512], bf16, tag="sq")
                            nc.vector.tensor_tensor(
                                sq[:, :], qraw[:, :], qraw[:, :], op=MUL)
                            var_b = ps_aux.tile([128, 512], f32,
                                                tag="qsw", name="var_b")
                            nc.tensor.matmul(
                                var_b[:, :], onesm_sb[:, :], sq[:, :],
                                start=True, stop=True)
                            nc.vector.tensor_copy(
                                var_all[:, len(fin), :], var_b[:, :])
                            qsw = ps_aux.tile([128, 512], f32, tag="qsw")
                            nc.tensor.matmul(
                                qsw[:, :], swapb_sb[:, :], qraw[:, :],
                                start=True, stop=True)
                            ct_t = tab_sb[:, 0 if ch < 4 else 2, :]
                            st_t = tab_sb[:, 1 if ch < 4 else 3, :]
                            t1 = t_pool.tile([128, 512], bf16, tag="t1",
                                             bufs=7)
                            t2 = t_pool.tile([128, 512], bf16, tag="t2")
                            nc.vector.tensor_tensor(
                                t1[:, :], qraw[:, :], ct_t, op=MUL)
                            nc.vector.tensor_tensor(
                                t2[:, :], qsw[:, :], st_t, op=MUL)
                            nc.vector.tensor_tensor(
                                t1[:, :], t1[:, :], t2[:, :], op=ADD)
                            dst = (qT_q[qt][:, ch, :] if ch < 4
                                   else kT_q[qt][:, :])
                            fin.append((t1, dst))
                        drain((0, 2, 4, 3)[qt])
                    # batched rms finish: the whole quarter's sqrt is ONE
                    # ACT instruction (the scheduler cannot split it into
                    # the Tanh/Exp stream, bounding table reloads to 2/qt)
                    sd_all = t_pool.tile([128, 5, 512], bf16, tag="sd",
                                         name="sd_all")
                    nc.scalar.activation(
                        sd_all[:, :, :], var_all[:, :, :], Act.Sqrt,
                        bias=eps_sb[:, :], scale=float(1.0 / D))
                    rr_all = t_pool.tile([128, 5, 512], bf16, tag="rr",
                                         name="rr_all")
                    nc.vector.reciprocal(rr_all[:, :, :], sd_all[:, :, :])
                    for i, (t1, dst) in enumerate(fin):
                        nc.vector.tensor_tensor(dst, t1[:, :],
                                                rr_all[:, i, :], op=MUL)
                # drain leftover attention work (j2 tail)
                drain(len(work))

            # ======== phase B: attention j2(h2,h3) + j3 + out-projections =
            with (
                tc.tile_pool(name="bst", bufs=1, space="PSUM") as st2_pool,
                tc.tile_pool(name="bctx", bufs=1, space="PSUM") as ps_ctx2,
                tc.tile_pool(name="bpo", bufs=3, space="PSUM") as po_pool,
                tc.tile_pool(name="btr", bufs=1, space="PSUM") as tr_pool,
            ):
                pending.extend((ec, jj) for jj in range(2) for ec in range(EC))

                def emit3():
                    if pending:
                        emit_oproj(po_pool, "po")
                    if len(pending) > 20:
                        emit_oproj(po_pool, "po")

                for j, h in [(2, 2), (2, 3)] + [(3, h) for h in range(G)]:
                    for pair in pairs_of(j):
                        score_pair(j, h, pair, st2_pool, emit=emit3)
                    pv_norm(j, h, ps_ctx2, tr_pool, "btr")
                    if (j, h) == (2, 3):
                        pending.extend((ec, 2) for ec in range(EC))
                pending.extend((ec, 3) for ec in range(EC))
            # tail: attention pools released, drain with deeper psum
            with tc.tile_pool(name="p3ps", bufs=4, space="PSUM") as ps3:
                while pending:
                    emit_oproj(ps3, "po3")
            if debug:
                for t in range(NQ):
                    nc.sync.dma_start(dbg_q[:, :, t*512:(t+1)*512], qT_q[t][:, :, :])
                    nc.sync.dma_start(dbg_k[:, t*512:(t+1)*512], kT_q[t][:, :])
                    nc.sync.dma_start(dbg_v[:, t*4:(t+1)*4, :], v_q[t][:, :, :])
                nc.sync.dma_start(dbg_ctx[:, :, :], ctx_sb[:, :, :])

    nc.compile()
    return nc


def _host_tables(positions_b, scale_vec):
    """cos/sin tables in [d, s] layout with norm-scale folded in, signed sin."""
    half = D // 2
    inv_freq = (1.0 / (THETA ** (np.arange(half, dtype=np.float32) / half))
                ).astype(np.float32)
    ang = positions_b.astype(np.float32)[:, None] * inv_freq[None, :]  # [S,64]
    cos = np.cos(ang).astype(np.float32)  # [S, 64]
    sin = np.sin(ang).astype(np.float32)
    sc = scale_vec.astype(np.float32)
    ct = np.empty((128, S), np.float32)
    st = np.empty((128, S), np.float32)
    ct[:half] = (cos * sc[None, :half]).T
    ct[half:] = (cos * sc[None, half:]).T
    st[:half] = (-sin * sc[None, half:]).T
    st[half:] = (sin * sc[None, :half]).T
    return ct, st


def _host_masks():
    m = np.zeros((8, 128, 512), np.float32)
    ki = np.arange(128)[:, None]
    qf = np.arange(512)[None, :]
    for d0, idx in _D0_MASK_IDX.items():
        dist = 128 * d0 + qf - ki
        m[idx] = ((dist >= 0) & (dist < WIN)).astype(np.float32)
    import ml_dtypes
    return m.astype(ml_dtypes.bfloat16)


_NC_CACHE = {}


def _get_module(nrep=1, debug=False):
    key = f"nc{nrep}_{debug}"
    if key not in _NC_CACHE:
        _NC_CACHE[key] = _build_module(nrep, debug=debug)
    return _NC_CACHE[key]


def _core_inputs(x, positions, Wq, Wk, Wv, Wo, q_norm_scale, k_norm_scale):
    import ml_dtypes
    bf = ml_dtypes.bfloat16
    masks_np = _host_masks()
    identb_np = np.eye(128, dtype=np.float32).astype(bf)
    onesm_np = np.ones((128, 128), np.float32).astype(bf)
    swapb_np = np.roll(np.eye(128, dtype=np.float32), 64, axis=0).astype(bf)

    per_b = {}
    for b in range(B):
        xT_np = np.ascontiguousarray(x[b].T).reshape(EC, 128, S).astype(bf)
        ctq_np, stq_np = _host_tables(positions[b], q_norm_scale)
        ctk_np, stk_np = _host_tables(positions[b], k_norm_scale)
        tabs_np = np.stack([ctq_np, stq_np, ctk_np, stk_np]).astype(bf)
        per_b[b] = (xT_np, tabs_np)

    in_maps = []
    for c in range(N_CORES):
        b, kv = c // KV, c % KV
        xT_np, tabs_np = per_b[b]
        wq_slice = Wq[:, kv * G:(kv + 1) * G, :].reshape(E, G * D)
        wk_slice = Wk[:, kv, :]
        wv_slice = Wv[:, kv, :]
        wqkv_np = (np.concatenate([wq_slice, wk_slice, wv_slice], axis=1)
                   .reshape(EC, 128, 768).astype(bf))
        wo_np = np.ascontiguousarray(Wo[kv * G:(kv + 1) * G]).astype(bf)
        in_maps.append({
            "xT": xT_np, "wqkv": wqkv_np, "wo": wo_np, "tabs": tabs_np,
            "masks": masks_np, "identb": identb_np, "swapb": swapb_np,
            "onesm": onesm_np,
        })
    return in_maps


def kernel(x, positions, mask, Wq, Wk, Wv, Wo, q_norm_scale, k_norm_scale,
           **_unused):
    from concourse import bass_utils

    x = np.asarray(x, np.float32)
    positions = np.asarray(positions)
    Wq = np.asarray(Wq, np.float32)
    Wk = np.asarray(Wk, np.float32)
    Wv = np.asarray(Wv, np.float32)
    Wo = np.asarray(Wo, np.float32)
    q_norm_scale = np.asarray(q_norm_scale, np.float32)
    k_norm_scale = np.asarray(k_norm_scale, np.float32)

    nc = _get_module()
    in_maps = _core_inputs(x, positions, Wq, Wk, Wv, Wo,
                           q_norm_scale, k_norm_scale)
    res = bass_utils.run_bass_kernel_spmd(
        nc, in_maps, core_ids=list(range(N_CORES)))
    out = np.zeros((B, S, E), np.float32)
    for c in range(N_CORES):
        b = c // KV
        out[b] += res.results[c]["outT"].astype(np.float32).T
    return out
